# revision 1
# baseline (speedup 1.0000x reference)
"""Trainium2 Bass kernel for nn_Encoder_85942295593405 (GNN message passing).

Math (reference):
  emb  = spikes @ W_emb + b_emb                      [b,t,N,D]
  send = relu(relu(emb@Ws1+bs1)@Ws2+bs2)             [b,t,N,D]
  recv = relu(relu(emb@Wr1+br1)@Wr2+br2)             [b,t,N,D]
  full = [send[:,1:,se]|pe[1:]|recv[:,:-1,re]|pe[:-1]]   [b,t-1,E,288]
  out  = relu(full@Wc1+bc1)@Wc2 + bc2                [b,t-1,E,5]

Key factorization: the edge gather commutes with the (linear) first combine
layer, so compute per-node tables Xs[t] = send[t]@Wc1[0:128,:] and
Xr[t] = recv[t]@Wc1[144:272,:] + pet[t] (N=128 rows instead of E=1024;
pet[t] is the t-dependent positional-encoding contribution, folded in via a
rank-1 K=1 matmul). The edge gather + the send/recv add then run as one-hot
gather-matmuls accumulated in PSUM (gather + add fused on the TensorEngine;
the one-hot matrices are exact in bf16), followed by ReLU (split across the
Scalar and Vector engines) and the thin @Wc2 output matmul. b_emb is folded
into bs1/br1 on the host (the embed layer has no activation); bc2 is added
on the host.

All heavy matmuls run in bf16 (4x the fp32 rate on the PE: fp32 lowers to
two half-rate passes); PSUM accumulation stays fp32.

Sharding: 8 cores = 2 batches x 4 time chunks. Each core computes 64
output timesteps (chunk starts [0,64,128,191]; the last chunk overlaps the
third by one step so all cores run an identical program).
"""

import os
import sys

import numpy as np

sys.path.insert(0, "/opt/trn_rl_repo")

import concourse.bass as bass  # noqa: E402
import concourse.mybir as mybir  # noqa: E402
import concourse.tile as tile  # noqa: E402
from concourse.bass_utils import run_bass_kernel_spmd  # noqa: E402

B, T, N, F = 2, 256, 128, 16
D, H, E = 128, 288, 1024
PE_SIZE = 16
NCORES = 8
TCHUNK = 64            # output timesteps per core
TSTEPS = TCHUNK + 1    # node-level timesteps per core
ROWS = TSTEPS * N      # node-level rows per core (8320)
T_LOS = [0, 64, 128, 191]
FC = 96                # legacy feature chunk (unused on device)
FCS = [128, 128, 32]   # feature chunks of the 288-wide edge activations
FOFF = [0, 128, 256]   # chunk offsets
NFC = 3                # number of feature chunks
EC = 512               # edge chunk (moving-operand max for fp32)
NEC = E // EC

F32 = mybir.dt.float32
BF16 = mybir.dt.bfloat16

LAST_RESULTS = None    # BassKernelResults of the last run (for test harness)

_PROGRAM = None


def _build_program():
    nc = bass.Bass()

    def inp(name, shape):
        return nc.dram_tensor(name, shape, F32, kind="ExternalInput")

    def binp(name, shape):
        return nc.dram_tensor(name, shape, BF16, kind="ExternalInput")

    spk = binp("spk", [F, ROWS])         # spikes, feature-major, per-core slice
    w_emb = binp("w_emb", [F, D])
    ws1 = binp("ws1", [D, D])
    bs1 = inp("bs1", [D, 1])             # column (per-partition ACT bias)
    ws2 = binp("ws2", [D, D])
    bs2 = inp("bs2", [D, 1])
    wr1 = binp("wr1", [D, D])
    br1 = inp("br1", [D, 1])
    wr2 = binp("wr2", [D, D])
    br2 = inp("br2", [D, 1])
    wc1s = binp("wc1s", [D, H])          # Wc1[0:128, :]
    wc1r = binp("wc1r", [D, H])          # Wc1[144:272, :]
    gs = binp("gs", [N, E])              # one-hot send gather matrix
    gr = binp("gr", [N, E])              # one-hot recv gather matrix
    pet_flat = binp("pet_flat", [N, TSTEPS * H])  # pe@Wc1 + bc1 per t,
    # replicated across partitions so the DVE can add it during the Xr cast
    wc2 = binp("wc2", [D, NFC, 5])       # K-chunk fc in rows [0:FCS[fc]]

    outd = nc.dram_tensor("out", [TCHUNK, 5, E], F32, kind="ExternalOutput")

    relu = mybir.ActivationFunctionType.Relu

    with tile.TileContext(nc) as tc:
        with tc.tile_pool(name="wpool", bufs=1) as wp:
            w_emb_sb = wp.tile([F, D], BF16, tag="w_emb")
            ws1_sb = wp.tile([D, D], BF16, tag="ws1")
            bs1_sb = wp.tile([D, 1], F32, tag="bs1")
            ws2_sb = wp.tile([D, D], BF16, tag="ws2")
            bs2_sb = wp.tile([D, 1], F32, tag="bs2")
            wr1_sb = wp.tile([D, D], BF16, tag="wr1")
            br1_sb = wp.tile([D, 1], F32, tag="br1")
            wr2_sb = wp.tile([D, D], BF16, tag="wr2")
            br2_sb = wp.tile([D, 1], F32, tag="br2")
            wc1s_sb = wp.tile([D, H], BF16, tag="wc1s")
            wc1r_sb = wp.tile([D, H], BF16, tag="wc1r")
            gs_sb = wp.tile([N, E], BF16, tag="gs")
            gr_sb = wp.tile([N, E], BF16, tag="gr")
            pet_sb = wp.tile([N, TSTEPS * H], BF16, tag="pet_flat")
            wc2_sb = wp.tile([D, NFC, 5], BF16, tag="wc2")
            # per-timestep node tables of the factored combine layer:
            # Xs[t] = send[t] @ Wc1[0:128]                          (bf16)
            # Xr[t] = recv[t] @ Wc1[144:272] + pet[t]
            # Split into 4 sub-tiles so stage B can start on early timesteps
            # while stage A is still producing later ones (deps are per-tile).
            tb = [0, 17, 33, 49, TSTEPS]
            xs_tbl = [wp.tile([N, (tb[p + 1] - tb[p]) * H], BF16,
                              name=f"xs_tbl{p}", tag=f"xs_tbl{p}")
                      for p in range(4)]
            xr_tbl = [wp.tile([N, (tb[p + 1] - tb[p]) * H], BF16,
                              name=f"xr_tbl{p}", tag=f"xr_tbl{p}")
                      for p in range(4)]

            def tbl_slice(tbl, t, off, w):
                p = 0
                while t >= tb[p + 1]:
                    p += 1
                return tbl[p][:, (t - tb[p]) * H + off:
                              (t - tb[p]) * H + off + w]

            for sb_t, dr_t in [
                (w_emb_sb, w_emb),
                (ws1_sb, ws1), (bs1_sb, bs1), (ws2_sb, ws2), (bs2_sb, bs2),
                (wr1_sb, wr1), (br1_sb, br1), (wr2_sb, wr2), (br2_sb, br2),
                (wc1s_sb, wc1s), (wc1r_sb, wc1r), (gs_sb, gs), (gr_sb, gr),
                (pet_sb, pet_flat), (wc2_sb, wc2),
            ]:
                nc.sync.dma_start(sb_t[:], dr_t[:])

            # ---- Stage A: node MLPs (feature-major) + Xs/Xr tables ----
            chunks = []
            r0 = 0
            while r0 < ROWS:
                ch = min(512, ROWS - r0)
                chunks.append((r0, ch))
                r0 += ch

            with (
                tc.tile_pool(name="stA_ps", bufs=2, space="PSUM") as psA,
                tc.tile_pool(name="x_ps", bufs=2, space="PSUM") as xps,
                tc.tile_pool(name="stA_sb", bufs=6) as sbA,
            ):
                for r0, ch in chunks:
                    spk_c = sbA.tile([F, ch], BF16, tag="spk")
                    nc.sync.dma_start(spk_c[:], spk[:, r0:r0 + ch])

                    emb_ps = psA.tile([D, ch], F32, tag="emb_ps")
                    nc.tensor.matmul(emb_ps[:], w_emb_sb[:], spk_c[:])
                    emb_sb = sbA.tile([D, ch], BF16, tag="emb_sb")
                    nc.vector.tensor_copy(emb_sb[:], emb_ps[:])

                    s1_ps = psA.tile([D, ch], F32, tag="mid")
                    nc.tensor.matmul(s1_ps[:], ws1_sb[:], emb_sb[:])
                    s1_sb = sbA.tile([D, ch], BF16, tag="s1_sb")
                    nc.scalar.activation(s1_sb[:], s1_ps[:], relu,
                                         bias=bs1_sb[:, 0:1])
                    s2_ps = psA.tile([D, ch], F32, tag="out2")
                    nc.tensor.matmul(s2_ps[:], ws2_sb[:], s1_sb[:])
                    send_c = sbA.tile([D, ch], BF16, tag="send_c")
                    nc.scalar.activation(send_c[:], s2_ps[:], relu,
                                         bias=bs2_sb[:, 0:1])

                    r1_ps = psA.tile([D, ch], F32, tag="mid")
                    nc.tensor.matmul(r1_ps[:], wr1_sb[:], emb_sb[:])
                    r1_sb = sbA.tile([D, ch], BF16, tag="r1_sb")
                    nc.scalar.activation(r1_sb[:], r1_ps[:], relu,
                                         bias=br1_sb[:, 0:1])
                    r2_ps = psA.tile([D, ch], F32, tag="out2")
                    nc.tensor.matmul(r2_ps[:], wr2_sb[:], r1_sb[:])
                    recv_c = sbA.tile([D, ch], BF16, tag="recv_c")
                    nc.scalar.activation(recv_c[:], r2_ps[:], relu,
                                         bias=br2_sb[:, 0:1])

                    for k in range(ch // N):
                        t = r0 // N + k
                        xr_ps = xps.tile([N, H], F32, tag="x_ps")
                        nc.tensor.matmul(
                            xr_ps[:], recv_c[:, k * N:(k + 1) * N], wc1r_sb[:])
                        xs_ps = xps.tile([N, H], F32, tag="x_ps")
                        nc.tensor.matmul(
                            xs_ps[:], send_c[:, k * N:(k + 1) * N], wc1s_sb[:])
                        nc.vector.tensor_copy(
                            tbl_slice(xs_tbl, t, 0, H), xs_ps[:])
                        nc.vector.tensor_add(
                            tbl_slice(xr_tbl, t, 0, H), xr_ps[:],
                            pet_sb[:, t * H:(t + 1) * H])

            # ---- Stage B: gather + relu + output layer, per (t_out, ec) ----
            with (
                tc.tile_pool(name="pre_ps", bufs=6, space="PSUM") as pps,
                tc.tile_pool(name="out_ps", bufs=2, space="PSUM") as ops,
                tc.tile_pool(name="stB_sb", bufs=12) as sbB,
            ):
                for i in range(TCHUNK):
                    hts = [[], []]
                    for fc in range(NFC):
                        w = FCS[fc]
                        off = FOFF[fc]
                        # both edge-chunks of this feature chunk, send side
                        # first — interleaving spaces the same-bank
                        # accumulate pairs so PSUM drains overlap fills.
                        pres = []
                        for ec in range(NEC):
                            pre = pps.tile([w, EC], F32, tag="pre",
                                           name=f"pre{fc}_{ec}")
                            nc.tensor.matmul(
                                pre[:],
                                tbl_slice(xs_tbl, i + 1, off, w),
                                gs_sb[:, ec * EC:(ec + 1) * EC],
                                start=True, stop=False)
                            pres.append(pre)
                        for ec in range(NEC):
                            nc.tensor.matmul(
                                pres[ec][:],
                                tbl_slice(xr_tbl, i, off, w),
                                gr_sb[:, ec * EC:(ec + 1) * EC],
                                start=False, stop=True)
                        for ec in range(NEC):
                            hT = sbB.tile([w, EC], BF16, tag="hT",
                                          name=f"hT{fc}_{ec}")
                            if fc == NFC - 1:
                                nc.vector.tensor_scalar_max(
                                    hT[:], pres[ec][:], 0.0)
                            else:
                                nc.scalar.activation(hT[:], pres[ec][:], relu)
                            hts[ec].append(hT)
                    o_ps_l = [ops.tile([5, EC], F32, tag="o_ps",
                                       name=f"o_ps{ec}") for ec in range(NEC)]
                    for fc in range(NFC):
                        for ec in range(NEC):
                            nc.tensor.matmul(o_ps_l[ec][:],
                                             wc2_sb[0:FCS[fc], fc, :],
                                             hts[ec][fc][:],
                                             start=(fc == 0),
                                             stop=(fc == NFC - 1))
                    for ec in range(NEC):
                        o_sb = sbB.tile([5, EC], F32, tag="o_sb")
                        nc.vector.tensor_copy(o_sb[:], o_ps_l[ec][:])
                        nc.sync.dma_start(
                            outd[i, :, ec * EC:(ec + 1) * EC], o_sb[:])

    _legalize_waits(nc)
    return nc


def _legalize_waits(nc):
    """Walrus codegen rejects instructions carrying more than one sync wait
    ("Too many sync wait commands", CoreV3GenImpl setupSyncWait). Hoist all
    but the last wait of any instruction onto standalone InstEventSemaphore
    instructions inserted just before it on the same engine queue —
    semantically identical, since waits execute in program order."""
    for f in nc.m.functions:
        for blk in f.blocks:
            insts = blk.instructions
            if not any(
                i.sync_info is not None and len(i.sync_info.on_wait or ()) > 1
                for i in insts
            ):
                continue
            out = []
            for inst in insts:
                si = inst.sync_info
                waits = list(si.on_wait) if si is not None and si.on_wait else []
                if len(waits) > 1:
                    for w in waits[:-1]:
                        out.append(mybir.InstEventSemaphore(
                            name=nc.get_next_instruction_name(),
                            engine=inst.engine,
                            ins=[],
                            outs=[],
                            sync_info=mybir.SyncInfo(on_wait=[w], on_update=[]),
                        ))
                    si.on_wait = waits[-1:]
                out.append(inst)
            blk.instructions = out


def _get_program():
    global _PROGRAM
    if _PROGRAM is None:
        _PROGRAM = _build_program()
    return _PROGRAM


def _wc2_chunks(Wc2):
    out = np.zeros((D, NFC, 5), np.float32)
    for fc in range(NFC):
        out[:FCS[fc], fc, :] = Wc2[FOFF[fc]:FOFF[fc] + FCS[fc]]
    return out


def _sinusoidal_pe(d, t):
    pos = np.arange(t, dtype=np.float32)[:, None]
    div = np.exp(np.arange(0, d, 2, dtype=np.float32)
                 * (-np.log(10000.0) / d)).astype(np.float32)
    pe = np.zeros((t, d), dtype=np.float32)
    pe[:, 0::2] = np.sin(pos * div)
    pe[:, 1::2] = np.cos(pos * div)
    return pe


def kernel(spikes, W_emb, b_emb, Ws1, bs1, Ws2, bs2, Wr1, br1, Wr2, br2,
           Wc1, bc1, Wc2, bc2, send_edges, recv_edges):
    global LAST_RESULTS
    f32 = np.float32
    spikes = np.asarray(spikes, f32)
    W_emb = np.ascontiguousarray(np.asarray(W_emb, f32))
    Wc1 = np.asarray(Wc1, f32)
    Wc2 = np.asarray(Wc2, f32)
    se = np.asarray(send_edges).astype(np.int64)
    re_ = np.asarray(recv_edges).astype(np.int64)

    # Positional-encoding contribution to the pre-ReLU combine activations:
    # pet_full[t_out] = pe[t_out+1] @ Wc1[128:144] + pe[t_out] @ Wc1[272:288]
    #                   + bc1, shape [T-1, 288].
    pe = _sinusoidal_pe(PE_SIZE, T)
    pet_full = (pe[1:] @ Wc1[D:D + PE_SIZE]
                + pe[:-1] @ Wc1[D + PE_SIZE + D:]
                + np.asarray(bc1, f32)[None, :]).astype(f32)

    nodes = np.arange(N, dtype=np.int64)
    G_send = (se[None, :] == nodes[:, None]).astype(f32)        # [N, E]
    G_recv = (re_[None, :] == nodes[:, None]).astype(f32)       # [N, E]

    import ml_dtypes
    bf16 = ml_dtypes.bfloat16
    # fold the (activation-free) embed bias into the first MLP-layer biases:
    # (emb + b_emb) @ W + b == emb @ W + (b + b_emb @ W)
    b_emb_v = np.asarray(b_emb, f32).reshape(1, D)
    bs1_f = np.asarray(bs1, f32) + (b_emb_v @ np.asarray(Ws1, f32))[0]
    br1_f = np.asarray(br1, f32) + (b_emb_v @ np.asarray(Wr1, f32))[0]
    common = dict(
        w_emb=W_emb.astype(bf16),
        ws1=np.ascontiguousarray(np.asarray(Ws1, f32)).astype(bf16),
        bs1=np.ascontiguousarray(bs1_f.reshape(D, 1)),
        ws2=np.ascontiguousarray(np.asarray(Ws2, f32)).astype(bf16),
        bs2=np.ascontiguousarray(np.asarray(bs2, f32).reshape(D, 1)),
        wr1=np.ascontiguousarray(np.asarray(Wr1, f32)).astype(bf16),
        br1=np.ascontiguousarray(br1_f.reshape(D, 1)),
        wr2=np.ascontiguousarray(np.asarray(Wr2, f32)).astype(bf16),
        br2=np.ascontiguousarray(np.asarray(br2, f32).reshape(D, 1)),
        wc1s=np.ascontiguousarray(Wc1[0:D]).astype(bf16),
        wc1r=np.ascontiguousarray(Wc1[D + PE_SIZE:D + PE_SIZE + D]).astype(bf16),
        gs=G_send.astype(bf16),
        gr=G_recv.astype(bf16),
        wc2=_wc2_chunks(Wc2).astype(bf16),
    )

    in_maps = []
    for core in range(NCORES):
        b = core // 4
        t_lo = T_LOS[core % 4]
        spk_slice = spikes[b, t_lo:t_lo + TSTEPS]               # [65,128,16]
        spkT = np.ascontiguousarray(
            spk_slice.reshape(ROWS, F).T).astype(bf16)          # [16, 8320]
        pet = np.zeros((TSTEPS, H), f32)
        pet[:TCHUNK] = pet_full[t_lo:t_lo + TCHUNK]
        pet_bc = np.ascontiguousarray(np.broadcast_to(
            pet.reshape(1, TSTEPS * H), (N, TSTEPS * H))).astype(bf16)
        in_maps.append(dict(common, spk=spkT, pet_flat=pet_bc))

    nc = _get_program()
    trace = bool(int(os.environ.get("KERNEL_TRACE", "0")))
    res = run_bass_kernel_spmd(nc, in_maps, list(range(NCORES)), trace=trace)
    LAST_RESULTS = res

    out = np.zeros((B, T - 1, E, 5), f32)
    for core in range(NCORES):
        b = core // 4
        t_lo = T_LOS[core % 4]
        r = res.results[core]["out"]                            # [64, 5, 1024]
        out[b, t_lo:t_lo + TCHUNK] = r.transpose(0, 2, 1)
    out += np.asarray(bc2, f32)[None, None, None, :]
    return out



# revision 7
# speedup vs baseline: 1.3625x; 1.3625x over previous
"""Trainium2 Bass kernel for nn_Encoder_85942295593405 (GNN message passing).

Math (reference):
  emb  = spikes @ W_emb + b_emb                      [b,t,N,D]
  send = relu(relu(emb@Ws1+bs1)@Ws2+bs2)             [b,t,N,D]
  recv = relu(relu(emb@Wr1+br1)@Wr2+br2)             [b,t,N,D]
  full = [send[:,1:,se]|pe[1:]|recv[:,:-1,re]|pe[:-1]]   [b,t-1,E,288]
  out  = relu(full@Wc1+bc1)@Wc2 + bc2                [b,t-1,E,5]

Factorizations:
  * The edge gather commutes with the first (linear) combine layer: per-node
    tables Xs[t] = send[t]@Wc1[0:128,:], Xr[t] = recv[t]@Wc1[144:272,:]
    (N=128 rows instead of E=1024). The gather+add runs as one-hot
    gather-matmuls accumulated in PSUM.
  * W_emb is folded into the first MLP layers on the host (the embed layer
    has no activation): Ws1' = W_emb@Ws1 (K=16), removing the embed matmul
    and its PSUM drain entirely.
  * The positional-encoding term pet[t] (pe@Wc1 slices + bc1) is a
    per-partition scalar in the edge-level layout (partition = feature), so
    it is applied for free as the bias of the stage-B ReLU drain.
  * The 32-wide tail of the 288 features is M-packed 4 timesteps at a time
    in the gather (full 128-wide stationary), and its output matmul uses a
    block-diagonal Wc2 [128,20]; the 128-wide chunks use zero-padded
    [128,20] Wc2 stationaries writing disjoint 5-row stripes of one PSUM
    accumulator, so a 4-timestep group needs 9 (not 12) 512-col matmuls per
    edge-chunk on both the gather and output stages.

All matmuls run in bf16 (fp32 is half rate); PSUM accumulation is fp32.
Stage A (node MLPs + tables) is interleaved with stage B (gather + combine)
at 4-timestep granularity so the TensorEngine never idles; PSUM is budgeted
exactly: 1 bank MLP + 2 banks X-tables + 4 banks gather + 1 bank output.
ReLU/copy drains rotate across the Scalar, Vector and GpSimd engines.

Sharding: 8 cores = 2 batches x 4 time chunks; each core produces 64 output
timesteps (starts [0,64,128,191]; chunk 3 overlaps chunk 2 by one step so
all cores run an identical program).
"""

import os
import sys

import numpy as np

sys.path.insert(0, "/opt/trn_rl_repo")

import concourse.bass as bass  # noqa: E402
import concourse.mybir as mybir  # noqa: E402
import concourse.tile as tile  # noqa: E402
from concourse.bass_utils import run_bass_kernel_spmd  # noqa: E402

B, T, N, F = 2, 256, 128, 16
D, H, E = 128, 288, 1024
PE_SIZE = 16
NCORES = 8
TCHUNK = 64            # output timesteps per core
TSTEPS = TCHUNK + 1    # node-level timesteps per core
ROWS = TSTEPS * N      # node-level rows per core (8320)
T_LOS = [0, 64, 128, 191]
EC = 512               # edge chunk (PSUM bank = 512 fp32)
NEC = E // EC
NG = TCHUNK // 4       # stage-B groups of 4 output timesteps
SUB = 256              # stage-A row sub-chunk (2 node timesteps)

F32 = mybir.dt.float32
BF16 = mybir.dt.bfloat16
RELU = mybir.ActivationFunctionType.Relu
ALU_ADD = mybir.AluOpType.add
ALU_MAX = mybir.AluOpType.max

USE_GPSIMD = False   # walrus: "GPSIMD Instructions cannot access PSUM"

LAST_RESULTS = None    # BassKernelResults of the last run (for test harness)
_PROGRAM = None


def _build_program():
    nc = bass.Bass()

    def inp(name, shape, dt=BF16):
        return nc.dram_tensor(name, shape, dt, kind="ExternalInput")

    spk = inp("spk", [F, ROWS])
    ws1p = inp("ws1p", [F, D])          # W_emb @ Ws1
    ws2 = inp("ws2", [D, D])
    wr1p = inp("wr1p", [F, D])          # W_emb @ Wr1
    wr2 = inp("wr2", [D, D])
    bs1 = inp("bs1", [D, 1], F32)
    bs2 = inp("bs2", [D, 1], F32)
    br1 = inp("br1", [D, 1], F32)
    br2 = inp("br2", [D, 1], F32)
    wc1s = inp("wc1s", [D, H])          # Wc1[0:128, :]
    wc1r = inp("wc1r", [D, H])          # Wc1[144:272, :]
    gs = inp("gs", [N, E])              # one-hot send gather matrix
    gr = inp("gr", [N, E])              # one-hot recv gather matrix
    wc2s = inp("wc2s", [D, 9, 20])      # 0: fc2 block-diag; 1+4*fc+tp: fc01
    pet0 = inp("pet0", [D, TCHUNK], F32)
    pet1 = inp("pet1", [D, TCHUNK], F32)
    pet2p = inp("pet2p", [D, NG], F32)

    outd = nc.dram_tensor("out", [NG, 2, 20, EC], F32, kind="ExternalOutput")

    with tile.TileContext(nc) as tc:
        with (
            tc.tile_pool(name="wpool", bufs=1) as wp,
            tc.tile_pool(name="ps", bufs=1, space="PSUM") as ps,
            tc.tile_pool(name="sbA", bufs=1) as sa,
            tc.tile_pool(name="sbB", bufs=1) as sbp,
        ):
            ws1p_sb = wp.tile([F, D], BF16, tag="ws1p")
            ws2_sb = wp.tile([D, D], BF16, tag="ws2")
            wr1p_sb = wp.tile([F, D], BF16, tag="wr1p")
            wr2_sb = wp.tile([D, D], BF16, tag="wr2")
            bs1_sb = wp.tile([D, 1], F32, tag="bs1")
            bs2_sb = wp.tile([D, 1], F32, tag="bs2")
            br1_sb = wp.tile([D, 1], F32, tag="br1")
            br2_sb = wp.tile([D, 1], F32, tag="br2")
            wc1s_sb = wp.tile([D, H], BF16, tag="wc1s")
            wc1r_sb = wp.tile([D, H], BF16, tag="wc1r")
            gs_sb = wp.tile([N, E], BF16, tag="gs")
            gr_sb = wp.tile([N, E], BF16, tag="gr")
            wc2s_sb = wp.tile([D, 9, 20], BF16, tag="wc2s")
            pet0_sb = wp.tile([D, TCHUNK], F32, tag="pet0")
            pet1_sb = wp.tile([D, TCHUNK], F32, tag="pet1")
            pet2p_sb = wp.tile([D, NG], F32, tag="pet2p")
            zeros_sb = wp.tile([N, EC], BF16, tag="zeros")
            # node tables: per side, 128-wide fc0/fc1 chunks and 32-wide fc2
            tbl_as = wp.tile([N, TSTEPS * 256], BF16, tag="tbl_as")
            tbl_bs = wp.tile([N, TSTEPS * 32], BF16, tag="tbl_bs")
            tbl_ar = wp.tile([N, TSTEPS * 256], BF16, tag="tbl_ar")
            tbl_br = wp.tile([N, TSTEPS * 32], BF16, tag="tbl_br")

            for sb_t, dr_t in [
                (ws1p_sb, ws1p), (ws2_sb, ws2), (wr1p_sb, wr1p),
                (wr2_sb, wr2), (bs1_sb, bs1), (bs2_sb, bs2), (br1_sb, br1),
                (br2_sb, br2), (wc1s_sb, wc1s), (wc1r_sb, wc1r),
                (gs_sb, gs), (gr_sb, gr), (wc2s_sb, wc2s),
                (pet0_sb, pet0), (pet1_sb, pet1), (pet2p_sb, pet2p),
            ]:
                nc.sync.dma_start(sb_t[:], dr_t[:])
            nc.vector.memset(zeros_sb[:], 0.0)

            # drain-engine rotation (PSUM -> SBUF relu/copy work)
            engines = ["act", "dve"] + (["gp"] if USE_GPSIMD else [])
            rot = [0]

            def drain_relu(hT, pre, pet_ap):
                e = engines[rot[0] % len(engines)]
                rot[0] += 1
                if e == "act":
                    nc.scalar.activation(hT, pre, RELU, bias=pet_ap)
                elif e == "dve":
                    nc.vector.scalar_tensor_tensor(
                        hT, pre, pet_ap, zeros_sb[:], ALU_ADD, ALU_MAX)
                else:
                    nc.gpsimd.scalar_tensor_tensor(
                        hT, pre, pet_ap, zeros_sb[:], ALU_ADD, ALU_MAX)

            def drain_copy(dst, src):
                e = engines[rot[0] % len(engines)]
                rot[0] += 1
                if e == "act":
                    nc.scalar.copy(dst, src)
                elif e == "dve":
                    nc.vector.tensor_copy(dst, src)
                else:
                    nc.gpsimd.tensor_copy(dst, src)

            # ---------- stage A ----------
            spk_tiles = {}

            def a_dma(i):
                """Prefetch chunk i's spikes (512 rows, 4 timesteps)."""
                def emit():
                    r0 = 512 * i
                    ch = min(512, ROWS - r0)
                    t_ = sa.tile([F, 512], BF16, tag="spk", bufs=2,
                                 name=f"spk_{i}")
                    nc.sync.dma_start(t_[:, 0:ch], spk[:, r0:r0 + ch])
                    spk_tiles[i] = t_
                return emit

            mlp_out = {}

            def a_mlp(i, sub, side):
                """One MLP sub-chunk (256 rows = 2 timesteps), one side."""
                def emit():
                    r0 = 512 * i + SUB * sub
                    ch = min(SUB, ROWS - r0)
                    if ch <= 0:
                        return
                    spk_c = spk_tiles[i][:, SUB * sub:SUB * sub + ch]
                    if side == "s":
                        w1, b1, w2, b2 = ws1p_sb, bs1_sb, ws2_sb, bs2_sb
                        ctag = "sendc"
                    else:
                        w1, b1, w2, b2 = wr1p_sb, br1_sb, wr2_sb, br2_sb
                        ctag = "recvc"
                    mlp_t = ps.tile([N, 512], F32, tag="mlp", bufs=1,
                                    name=f"mlp_{i}_{sub}_{side}")
                    nc.tensor.matmul(mlp_t[:, 0:ch], w1[:], spk_c)
                    h1 = sa.tile([D, SUB], BF16, tag=f"h1{side}", bufs=2,
                                 name=f"h1_{i}_{sub}_{side}")
                    nc.scalar.activation(h1[:, 0:ch], mlp_t[:, 0:ch], RELU,
                                         bias=b1[:, 0:1])
                    nc.tensor.matmul(mlp_t[:, 256:256 + ch], w2[:],
                                     h1[:, 0:ch])
                    out_c = sa.tile([D, SUB], BF16, tag=ctag, bufs=2,
                                    name=f"{ctag}_{i}_{sub}")
                    nc.scalar.activation(out_c[:, 0:ch],
                                         mlp_t[:, 256:256 + ch], RELU,
                                         bias=b2[:, 0:1])
                    mlp_out[(side, i, sub)] = out_c
                return emit

            def a_x(t, k, sendc, recvc):
                """X-table matmuls + drains for node timestep t (col k in
                the 256-row sub-chunk's tiles)."""
                for side, src, wc1, ta, tb in (
                    ("s", sendc, wc1s_sb, tbl_as, tbl_bs),
                    ("r", recvc, wc1r_sb, tbl_ar, tbl_br),
                ):
                    if side == "s" and t == 0:
                        continue
                    if side == "r" and t == TCHUNK:
                        continue
                    x_t = ps.tile([N, 512], F32, tag="x", bufs=2,
                                  name=f"x_{t}_{side}")
                    nc.tensor.matmul(
                        x_t[:, 0:H], src[:, k * N:(k + 1) * N], wc1[:])
                    drain_copy(tbl_slice(ta, t, 256), x_t[:, 0:256])
                    drain_copy(tbl_slice(tb, t, 32), x_t[:, 256:H])

            def tbl_slice(tbl, t, w):
                return tbl[:, t * w:(t + 1) * w]

            def a_units(i):
                """Stage-A emission units for chunk i (timesteps 4i..4i+3),
                plus the prefetch of chunk i+1."""
                units = []
                if i + 1 <= 16:
                    units.append(a_dma(i + 1))
                if i == 16:
                    units.append(a_mlp(i, 0, "s"))

                    def xs64():
                        a_x(TCHUNK, 0, mlp_out[("s", 16, 0)], None)
                    units.append(xs64)
                    return units
                for sub in range(2):
                    units.append(a_mlp(i, sub, "s"))
                    units.append(a_mlp(i, sub, "r"))

                    def xpair(sub=sub):
                        sc = mlp_out[("s", i, sub)]
                        rc = mlp_out[("r", i, sub)]
                        a_x(4 * i + 2 * sub, 0, sc, rc)
                        a_x(4 * i + 2 * sub + 1, 1, sc, rc)
                    units.append(xpair)
                return units

            # ---------- stage B ----------
            # slots per edge chunk: 0 = fc2 4-t-packed, 1.. = 2*tp+fc+1
            def slot_aps(j, s, ec):
                """(send_lhsT, recv_lhsT, moving_s, moving_r, pet_ap, widx)"""
                ms = gs_sb[:, ec * EC:(ec + 1) * EC]
                mr = gr_sb[:, ec * EC:(ec + 1) * EC]
                if s == 0:
                    ls = tbl_bs[:, (4 * j + 1) * 32:(4 * j + 5) * 32]
                    lr = tbl_br[:, (4 * j) * 32:(4 * j + 4) * 32]
                    pet = pet2p_sb[:, j:j + 1]
                    widx = 0
                else:
                    tp, fc = (s - 1) // 2, (s - 1) % 2
                    t = 4 * j + tp
                    ls = tbl_as[:, (t + 1) * 256 + fc * 128:
                                (t + 1) * 256 + fc * 128 + 128]
                    lr = tbl_ar[:, t * 256 + fc * 128:
                                t * 256 + fc * 128 + 128]
                    pet = (pet0_sb if fc == 0 else pet1_sb)[:, t:t + 1]
                    widx = 1 + 4 * fc + tp
                return ls, lr, ms, mr, pet, widx

            def b_units(j):
                state = {"o": None, "osb": None, "hT": {}, "wq": [],
                         "meta": {}}

                def gather_start(s, ec):
                    ls, lr, ms, mr, pet, widx = slot_aps(j, s, ec)
                    pre = ps.tile([N, 512], F32, tag="pre", bufs=4,
                                  name=f"pre_{j}_{ec}_{s}")
                    state["meta"][(s, ec)] = (pet, widx)
                    nc.tensor.matmul(pre[:], ls, ms, start=True, stop=False)
                    return (pre, lr, mr)

                def gather_fin(s, ec, pre, lr, mr):
                    nc.tensor.matmul(pre[:], lr, mr, start=False, stop=True)
                    hT = sbp.tile([N, EC], BF16, tag="hT", bufs=8,
                                  name=f"hT_{j}_{ec}_{s}")
                    pet, _ = state["meta"][(s, ec)]
                    drain_relu(hT[:], pre[:], pet)
                    state["hT"][(s, ec)] = hT
                    state["wq"].append((s, ec))

                def wc2_flush(keep):
                    while len(state["wq"]) > keep:
                        s, ec = state["wq"].pop(0)
                        _, widx = state["meta"][(s, ec)]
                        hT = state["hT"][(s, ec)]
                        off = 0 if ec == 0 else 32
                        nc.tensor.matmul(
                            state["o"][off:off + 20, :],
                            wc2s_sb[:, widx, :], hT[:],
                            start=(s == 0), stop=(s == 8))

                def gpair(sa_, sb_, ec):
                    def emit():
                        if state["o"] is None:
                            state["o"] = ps.tile([N, 512], F32, tag="o",
                                                 bufs=1, name=f"o_{j}")
                        g1 = gather_start(sa_, ec)
                        g2 = gather_start(sb_, ec) if sb_ is not None else None
                        gather_fin(sa_, ec, *g1)
                        if g2 is not None:
                            gather_fin(sb_, ec, *g2)
                        wc2_flush(4)
                    return emit

                def ec_tail(ec):
                    def emit():
                        wc2_flush(0)
                        if ec == 0:
                            state["osb"] = sbp.tile([52, EC], F32, tag="osb",
                                                    bufs=2, name=f"osb_{j}")
                            nc.vector.tensor_copy(
                                state["osb"][0:20, :], state["o"][0:20, :])
                            nc.sync.dma_start(outd[j, 0, :, :],
                                              state["osb"][0:20, :])
                        else:
                            nc.scalar.copy(
                                state["osb"][32:52, :], state["o"][32:52, :])
                            nc.sync.dma_start(outd[j, 1, :, :],
                                              state["osb"][32:52, :])
                    return emit

                units = []
                for ec in range(NEC):
                    units.append(gpair(0, 1, ec))
                    units.append(gpair(2, 3, ec))
                    units.append(gpair(4, 5, ec))
                    units.append(gpair(6, 7, ec))
                    units.append(gpair(8, None, ec))
                    units.append(ec_tail(ec))
                return units

            # ---------- riffled emission ----------
            a_dma(0)()
            for i in range(18):
                a = a_units(i) if i <= 16 else []
                b = b_units(i - 2) if i >= 2 else []
                if not a:
                    out_u = b
                elif not b:
                    out_u = a
                else:
                    # spread a-units evenly among b-units
                    na, nb = len(a), len(b)
                    out_u = []
                    ai = 0
                    for k, bu in enumerate(b):
                        out_u.append(bu)
                        want = (k + 1) * na // nb
                        while ai < want:
                            out_u.append(a[ai])
                            ai += 1
                    out_u.extend(a[ai:])
                for u in out_u:
                    u()

    _legalize_waits(nc)
    return nc


def _legalize_waits(nc):
    """Walrus codegen rejects instructions carrying more than one sync wait
    ("Too many sync wait commands", CoreV3GenImpl setupSyncWait). Hoist all
    but the last wait of any instruction onto standalone InstEventSemaphore
    instructions inserted just before it on the same engine queue —
    semantically identical, since waits execute in program order."""
    for f in nc.m.functions:
        for blk in f.blocks:
            insts = blk.instructions
            if not any(
                i.sync_info is not None and len(i.sync_info.on_wait or ()) > 1
                for i in insts
            ):
                continue
            out = []
            for inst in insts:
                si = inst.sync_info
                waits = list(si.on_wait) if si is not None and si.on_wait else []
                if len(waits) > 1:
                    for w in waits[:-1]:
                        out.append(mybir.InstEventSemaphore(
                            name=nc.get_next_instruction_name(),
                            engine=inst.engine,
                            ins=[],
                            outs=[],
                            sync_info=mybir.SyncInfo(on_wait=[w], on_update=[]),
                        ))
                    si.on_wait = waits[-1:]
                out.append(inst)
            blk.instructions = out


def _get_program():
    global _PROGRAM
    if _PROGRAM is None:
        _PROGRAM = _build_program()
    return _PROGRAM


def _sinusoidal_pe(d, t):
    pos = np.arange(t, dtype=np.float32)[:, None]
    div = np.exp(np.arange(0, d, 2, dtype=np.float32)
                 * (-np.log(10000.0) / d)).astype(np.float32)
    pe = np.zeros((t, d), dtype=np.float32)
    pe[:, 0::2] = np.sin(pos * div)
    pe[:, 1::2] = np.cos(pos * div)
    return pe


def kernel(spikes, W_emb, b_emb, Ws1, bs1, Ws2, bs2, Wr1, br1, Wr2, br2,
           Wc1, bc1, Wc2, bc2, send_edges, recv_edges):
    global LAST_RESULTS
    import ml_dtypes
    f32 = np.float32
    bf16 = ml_dtypes.bfloat16
    spikes = np.asarray(spikes, f32)
    W_emb = np.asarray(W_emb, f32)
    Wc1 = np.asarray(Wc1, f32)
    Wc2 = np.asarray(Wc2, f32)
    se = np.asarray(send_edges).astype(np.int64)
    re_ = np.asarray(recv_edges).astype(np.int64)

    # pet[t_out] = pe[t_out+1] @ Wc1[128:144] + pe[t_out] @ Wc1[272:288] + bc1
    pe = _sinusoidal_pe(PE_SIZE, T)
    pet_full = (pe[1:] @ Wc1[D:D + PE_SIZE]
                + pe[:-1] @ Wc1[D + PE_SIZE + D:]
                + np.asarray(bc1, f32)[None, :]).astype(f32)    # [T-1, 288]

    nodes = np.arange(N, dtype=np.int64)
    G_send = (se[None, :] == nodes[:, None]).astype(f32)        # [N, E]
    G_recv = (re_[None, :] == nodes[:, None]).astype(f32)       # [N, E]

    # fold the (activation-free) embed layer into the first MLP layers
    b_emb_v = np.asarray(b_emb, f32).reshape(1, D)
    Ws1f = np.asarray(Ws1, f32)
    Wr1f = np.asarray(Wr1, f32)
    bs1_f = np.asarray(bs1, f32) + (b_emb_v @ Ws1f)[0]
    br1_f = np.asarray(br1, f32) + (b_emb_v @ Wr1f)[0]

    # Wc2 stationaries [128, 9, 20]: slot 0 = fc2 block-diagonal
    # (row 32*tp+f -> col 5*tp+c); slots 1+4*fc+tp = Wc2[fc*128:...]
    # placed at column offset 5*tp, zero elsewhere.
    wc2s = np.zeros((D, 9, 20), f32)
    for tp in range(4):
        wc2s[32 * tp:32 * tp + 32, 0, 5 * tp:5 * tp + 5] = Wc2[256:288]
        for fc in range(2):
            wc2s[:, 1 + 4 * fc + tp, 5 * tp:5 * tp + 5] = \
                Wc2[fc * 128:(fc + 1) * 128]

    common = dict(
        ws1p=np.ascontiguousarray(W_emb @ Ws1f).astype(bf16),
        ws2=np.ascontiguousarray(np.asarray(Ws2, f32)).astype(bf16),
        wr1p=np.ascontiguousarray(W_emb @ Wr1f).astype(bf16),
        wr2=np.ascontiguousarray(np.asarray(Wr2, f32)).astype(bf16),
        bs1=np.ascontiguousarray(bs1_f.reshape(D, 1)),
        bs2=np.ascontiguousarray(np.asarray(bs2, f32).reshape(D, 1)),
        br1=np.ascontiguousarray(br1_f.reshape(D, 1)),
        br2=np.ascontiguousarray(np.asarray(br2, f32).reshape(D, 1)),
        wc1s=np.ascontiguousarray(Wc1[0:D]).astype(bf16),
        wc1r=np.ascontiguousarray(Wc1[D + PE_SIZE:D + PE_SIZE + D]).astype(bf16),
        gs=G_send.astype(bf16),
        gr=G_recv.astype(bf16),
        wc2s=np.ascontiguousarray(wc2s).astype(bf16),
    )

    in_maps = []
    for core in range(NCORES):
        b = core // 4
        t_lo = T_LOS[core % 4]
        spk_slice = spikes[b, t_lo:t_lo + TSTEPS]               # [65,128,16]
        spkT = np.ascontiguousarray(
            spk_slice.reshape(ROWS, F).T).astype(bf16)          # [16, 8320]
        pet = pet_full[t_lo:t_lo + TCHUNK]                      # [64, 288]
        pet0 = np.ascontiguousarray(pet[:, 0:128].T)            # [128, 64]
        pet1 = np.ascontiguousarray(pet[:, 128:256].T)
        pet2p = np.zeros((D, NG), f32)
        for j in range(NG):
            pet2p[:, j] = pet[4 * j:4 * j + 4, 256:288].reshape(128)
        in_maps.append(dict(common, spk=spkT, pet0=pet0, pet1=pet1,
                            pet2p=pet2p))

    nc = _get_program()
    trace = bool(int(os.environ.get("KERNEL_TRACE", "0")))
    res = run_bass_kernel_spmd(nc, in_maps, list(range(NCORES)), trace=trace)
    LAST_RESULTS = res

    out = np.zeros((B, T - 1, E, 5), f32)
    for core in range(NCORES):
        b = core // 4
        t_lo = T_LOS[core % 4]
        r = res.results[core]["out"]                         # [16, 2, 20, 512]
        r = r.reshape(NG, 2, 4, 5, EC).transpose(0, 2, 1, 4, 3)
        out[b, t_lo:t_lo + TCHUNK] = r.reshape(TCHUNK, E, 5)
    out += np.asarray(bc2, f32)[None, None, None, :]
    return out


# revision 17
# speedup vs baseline: 1.3996x; 1.0272x over previous
"""Trainium2 Bass kernel for nn_Encoder_85942295593405 (GNN message passing).

Math (reference):
  emb  = spikes @ W_emb + b_emb                      [b,t,N,D]
  send = relu(relu(emb@Ws1+bs1)@Ws2+bs2)             [b,t,N,D]
  recv = relu(relu(emb@Wr1+br1)@Wr2+br2)             [b,t,N,D]
  full = [send[:,1:,se]|pe[1:]|recv[:,:-1,re]|pe[:-1]]   [b,t-1,E,288]
  out  = relu(full@Wc1+bc1)@Wc2 + bc2                [b,t-1,E,5]

Factorizations:
  * The edge gather commutes with the first (linear) combine layer: per-node
    tables Xs[t] = send[t]@Wc1[0:128,:], Xr[t] = recv[t]@Wc1[144:272,:]
    (N=128 rows instead of E=1024). The gather+add runs as one-hot
    gather-matmuls accumulated in PSUM.
  * W_emb is folded into the first MLP layers on the host (the embed layer
    has no activation): Ws1' = W_emb@Ws1 (K=16), removing the embed matmul
    and its PSUM drain entirely.
  * The positional-encoding term pet[t] (pe@Wc1 slices + bc1) is a
    per-partition scalar in the edge-level layout (partition = feature), so
    it is applied for free as the bias of the stage-B ReLU drain.
  * The 32-wide tail of the 288 features is M-packed 4 timesteps at a time
    in the gather (full 128-wide stationary), and its output matmul uses a
    block-diagonal Wc2 [128,20]; the 128-wide chunks use zero-padded
    [128,20] Wc2 stationaries writing disjoint 5-row stripes of one PSUM
    accumulator, so a 4-timestep group needs 9 (not 12) 512-col matmuls per
    edge-chunk on both the gather and output stages.

All matmuls run in bf16 (fp32 is half rate); PSUM accumulation is fp32.
Stage A (node MLPs + tables) is interleaved with stage B (gather + combine)
at 4-timestep granularity so the TensorEngine never idles; PSUM is budgeted
exactly: 1 bank MLP + 2 banks X-tables + 4 banks gather + 1 bank output.
ReLU/copy drains rotate across the Scalar, Vector and GpSimd engines.

Sharding: 8 cores = 2 batches x 4 time chunks; each core produces 64 output
timesteps (starts [0,64,128,191]; chunk 3 overlaps chunk 2 by one step so
all cores run an identical program).
"""

import os
import sys

import numpy as np

sys.path.insert(0, "/opt/trn_rl_repo")

import concourse.bass as bass  # noqa: E402
import concourse.mybir as mybir  # noqa: E402
import concourse.tile as tile  # noqa: E402
from concourse.bass_utils import run_bass_kernel_spmd  # noqa: E402

B, T, N, F = 2, 256, 128, 16
D, H, E = 128, 288, 1024
PE_SIZE = 16
NCORES = 8
TCHUNK = 64            # output timesteps per core
TSTEPS = TCHUNK + 1    # node-level timesteps per core
ROWS = TSTEPS * N      # node-level rows per core (8320)
T_LOS = [0, 64, 128, 191]
EC = 512               # edge chunk (PSUM bank = 512 fp32)
NEC = E // EC
NG = TCHUNK // 4       # stage-B groups of 4 output timesteps
SUB = 256              # stage-A row sub-chunk (2 node timesteps)

F32 = mybir.dt.float32
BF16 = mybir.dt.bfloat16
RELU = mybir.ActivationFunctionType.Relu
ALU_ADD = mybir.AluOpType.add
ALU_MAX = mybir.AluOpType.max

USE_GPSIMD = False   # walrus: "GPSIMD Instructions cannot access PSUM"

LAST_RESULTS = None    # BassKernelResults of the last run (for test harness)
_PROGRAM = None


def _build_program():
    nc = bass.Bass()

    def inp(name, shape, dt=BF16):
        return nc.dram_tensor(name, shape, dt, kind="ExternalInput")

    spk = inp("spk", [F, ROWS])
    ws1p = inp("ws1p", [F, D])          # W_emb @ Ws1
    ws2 = inp("ws2", [D, D])
    wr1p = inp("wr1p", [F, D])          # W_emb @ Wr1
    wr2 = inp("wr2", [D, D])
    bs1 = inp("bs1", [D, 1], F32)
    bs2 = inp("bs2", [D, 1], F32)
    br1 = inp("br1", [D, 1], F32)
    br2 = inp("br2", [D, 1], F32)
    wc1s = inp("wc1s", [D, H])          # Wc1[0:128, :]
    wc1r = inp("wc1r", [D, H])          # Wc1[144:272, :]
    gs = inp("gs", [N, E])              # one-hot send gather matrix
    gr = inp("gr", [N, E])              # one-hot recv gather matrix
    wc2s = inp("wc2s", [D, 9, 20])      # 0: fc2 block-diag; 1+4*fc+tp: fc01
    pet0 = inp("pet0", [D, TCHUNK], F32)
    pet1 = inp("pet1", [D, TCHUNK], F32)
    pet2p = inp("pet2p", [D, NG], F32)

    outd = nc.dram_tensor("out", [NG, 2, 20, EC], F32, kind="ExternalOutput")

    with tile.TileContext(nc) as tc:
        with (
            tc.tile_pool(name="wpool", bufs=1) as wp,
            tc.tile_pool(name="ps", bufs=1, space="PSUM") as ps,
            tc.tile_pool(name="sbA", bufs=1) as sa,
            tc.tile_pool(name="sbB", bufs=1) as sbp,
        ):
            ws1p_sb = wp.tile([F, D], BF16, tag="ws1p")
            ws2_sb = wp.tile([D, D], BF16, tag="ws2")
            wr1p_sb = wp.tile([F, D], BF16, tag="wr1p")
            wr2_sb = wp.tile([D, D], BF16, tag="wr2")
            bs1_sb = wp.tile([D, 1], F32, tag="bs1")
            bs2_sb = wp.tile([D, 1], F32, tag="bs2")
            br1_sb = wp.tile([D, 1], F32, tag="br1")
            br2_sb = wp.tile([D, 1], F32, tag="br2")
            wc1s_sb = wp.tile([D, H], BF16, tag="wc1s")
            wc1r_sb = wp.tile([D, H], BF16, tag="wc1r")
            gs_sb = wp.tile([N, E], BF16, tag="gs")
            gr_sb = wp.tile([N, E], BF16, tag="gr")
            wc2s_sb = wp.tile([D, 9, 20], BF16, tag="wc2s")
            pet0_sb = wp.tile([D, TCHUNK], F32, tag="pet0")
            pet1_sb = wp.tile([D, TCHUNK], F32, tag="pet1")
            pet2p_sb = wp.tile([D, NG], F32, tag="pet2p")
            zeros_sb = wp.tile([N, EC], BF16, tag="zeros")
            # node tables: per side, 128-wide fc0/fc1 chunks and 32-wide fc2
            tbl_as = wp.tile([N, TSTEPS * 256], BF16, tag="tbl_as")
            tbl_bs = wp.tile([N, TSTEPS * 32], BF16, tag="tbl_bs")
            tbl_ar = wp.tile([N, TSTEPS * 256], BF16, tag="tbl_ar")
            tbl_br = wp.tile([N, TSTEPS * 32], BF16, tag="tbl_br")

            # Input DMAs issued from otherwise-idle engine queues, ordered by
            # first use (a DMA trigger costs ~650ns of issuing-queue time, so
            # a single queue would delay the first matmul by ~10us).
            nc.gpsimd.dma_start(ws1p_sb[:], ws1p[:])
            # (spk chunk 0 is issued next on the gpsimd queue, below)
            for sb_t, dr_t in [
                (bs1_sb, bs1), (bs2_sb, bs2), (br1_sb, br1), (br2_sb, br2),
            ]:
                nc.scalar.dma_start(sb_t[:], dr_t[:])
            for sb_t, dr_t in [
                (wc1s_sb, wc1s), (wc1r_sb, wc1r), (pet0_sb, pet0),
                (pet1_sb, pet1), (pet2p_sb, pet2p), (gs_sb, gs),
                (gr_sb, gr), (wc2s_sb, wc2s),
            ]:
                nc.sync.dma_start(sb_t[:], dr_t[:])
            nc.vector.memset(zeros_sb[:], 0.0)

            # drain-engine rotation (PSUM -> SBUF relu/copy work)
            engines = ["act", "dve"] + (["gp"] if USE_GPSIMD else [])
            rot = [0]

            def drain_relu(hT, pre, pet_ap):
                e = engines[rot[0] % len(engines)]
                rot[0] += 1
                if e == "act":
                    nc.scalar.activation(hT, pre, RELU, bias=pet_ap)
                elif e == "dve":
                    nc.vector.scalar_tensor_tensor(
                        hT, pre, pet_ap, zeros_sb[:], ALU_ADD, ALU_MAX)
                else:
                    nc.gpsimd.scalar_tensor_tensor(
                        hT, pre, pet_ap, zeros_sb[:], ALU_ADD, ALU_MAX)

            def drain_copy(dst, src):
                e = engines[rot[0] % len(engines)]
                rot[0] += 1
                if e == "act":
                    nc.scalar.copy(dst, src)
                elif e == "dve":
                    nc.vector.tensor_copy(dst, src)
                else:
                    nc.gpsimd.tensor_copy(dst, src)

            # ---------- stage A ----------
            spk_tiles = {}

            def a_dma(i):
                """Prefetch chunk i's spikes (512 rows, 4 timesteps)."""
                def emit():
                    r0 = 512 * i
                    ch = min(512, ROWS - r0)
                    t_ = sa.tile([F, 512], BF16, tag="spk", bufs=2,
                                 name=f"spk_{i}")
                    nc.gpsimd.dma_start(t_[:, 0:ch], spk[:, r0:r0 + ch])
                    spk_tiles[i] = t_
                return emit

            mlp_out = {}

            def a_mlp(i, sub, side):
                """One MLP sub-chunk (256 rows = 2 timesteps), one side."""
                def emit():
                    r0 = 512 * i + SUB * sub
                    ch = min(SUB, ROWS - r0)
                    if ch <= 0:
                        return
                    spk_c = spk_tiles[i][:, SUB * sub:SUB * sub + ch]
                    if side == "s":
                        w1, b1, w2, b2 = ws1p_sb, bs1_sb, ws2_sb, bs2_sb
                        ctag = "sendc"
                    else:
                        w1, b1, w2, b2 = wr1p_sb, br1_sb, wr2_sb, br2_sb
                        ctag = "recvc"
                    # iterations 0-1 run before stage B starts: borrow its
                    # idle PSUM banks so MLP/X matmuls don't wait on drains
                    if i < 2:
                        mtag = "o" if (sub + (side == "r")) % 2 else "mlp"
                    else:
                        mtag = "mlp"
                    mlp_t = ps.tile([N, 512], F32, tag=mtag, bufs=1,
                                    name=f"mlp_{i}_{sub}_{side}")
                    nc.tensor.matmul(mlp_t[:, 0:ch], w1[:], spk_c)
                    h1 = sa.tile([D, SUB], BF16, tag=f"h1{side}", bufs=2,
                                 name=f"h1_{i}_{sub}_{side}")
                    nc.scalar.activation(h1[:, 0:ch], mlp_t[:, 0:ch], RELU,
                                         bias=b1[:, 0:1])
                    nc.tensor.matmul(mlp_t[:, 256:256 + ch], w2[:],
                                     h1[:, 0:ch])
                    out_c = sa.tile([D, SUB], BF16, tag=ctag, bufs=2,
                                    name=f"{ctag}_{i}_{sub}")
                    nc.scalar.activation(out_c[:, 0:ch],
                                         mlp_t[:, 256:256 + ch], RELU,
                                         bias=b2[:, 0:1])
                    mlp_out[(side, i, sub)] = out_c
                return emit

            def a_x(t, k, sendc, recvc, early=False):
                """X-table matmuls + drains for node timestep t (col k in
                the 256-row sub-chunk's tiles)."""
                for side, src, wc1, ta, tb in (
                    ("s", sendc, wc1s_sb, tbl_as, tbl_bs),
                    ("r", recvc, wc1r_sb, tbl_ar, tbl_br),
                ):
                    if side == "s" and t == 0:
                        continue
                    if side == "r" and t == TCHUNK:
                        continue
                    xtag, xbufs = ("pre", 4) if early else ("x", 2)
                    x_t = ps.tile([N, 512], F32, tag=xtag, bufs=xbufs,
                                  name=f"x_{t}_{side}")
                    nc.tensor.matmul(
                        x_t[:, 0:H], src[:, k * N:(k + 1) * N], wc1[:])
                    drain_copy(tbl_slice(ta, t, 256), x_t[:, 0:256])
                    drain_copy(tbl_slice(tb, t, 32), x_t[:, 256:H])

            def tbl_slice(tbl, t, w):
                return tbl[:, t * w:(t + 1) * w]

            def a_units(i):
                """Stage-A emission units for chunk i (timesteps 4i..4i+3),
                plus the prefetch of chunk i+1."""
                units = []
                if i + 1 <= 16:
                    units.append(a_dma(i + 1))
                if i == 16:
                    units.append(a_mlp(i, 0, "s"))

                    def xs64():
                        a_x(TCHUNK, 0, mlp_out[("s", 16, 0)], None)
                    units.append(xs64)
                    return units
                for sub in range(2):
                    units.append(a_mlp(i, sub, "s"))
                    units.append(a_mlp(i, sub, "r"))

                    def xpair(sub=sub):
                        sc = mlp_out[("s", i, sub)]
                        rc = mlp_out[("r", i, sub)]
                        a_x(4 * i + 2 * sub, 0, sc, rc, early=(i < 2))
                        a_x(4 * i + 2 * sub + 1, 1, sc, rc, early=(i < 2))
                    units.append(xpair)
                return units

            # ---------- stage B ----------
            # slots per edge chunk: 0 = fc2 4-t-packed, 1.. = 2*tp+fc+1
            def slot_aps(j, s, ec):
                """(send_lhsT, recv_lhsT, moving_s, moving_r, pet_ap, widx)"""
                ms = gs_sb[:, ec * EC:(ec + 1) * EC]
                mr = gr_sb[:, ec * EC:(ec + 1) * EC]
                if s == 0:
                    ls = tbl_bs[:, (4 * j + 1) * 32:(4 * j + 5) * 32]
                    lr = tbl_br[:, (4 * j) * 32:(4 * j + 4) * 32]
                    pet = pet2p_sb[:, j:j + 1]
                    widx = 0
                else:
                    tp, fc = (s - 1) // 2, (s - 1) % 2
                    t = 4 * j + tp
                    ls = tbl_as[:, (t + 1) * 256 + fc * 128:
                                (t + 1) * 256 + fc * 128 + 128]
                    lr = tbl_ar[:, t * 256 + fc * 128:
                                t * 256 + fc * 128 + 128]
                    pet = (pet0_sb if fc == 0 else pet1_sb)[:, t:t + 1]
                    widx = 1 + 4 * fc + tp
                return ls, lr, ms, mr, pet, widx

            def b_units(j):
                state = {"o": None, "osb": None, "hT": {}, "wq": [],
                         "meta": {}}

                def gather_start(s, ec):
                    ls, lr, ms, mr, pet, widx = slot_aps(j, s, ec)
                    pre = ps.tile([N, 512], F32, tag="pre", bufs=4,
                                  name=f"pre_{j}_{ec}_{s}")
                    state["meta"][(s, ec)] = (pet, widx)
                    nc.tensor.matmul(pre[:], ls, ms, start=True, stop=False)
                    return (pre, lr, mr)

                def gather_fin(s, ec, pre, lr, mr):
                    nc.tensor.matmul(pre[:], lr, mr, start=False, stop=True)
                    hT = sbp.tile([N, EC], BF16, tag="hT", bufs=8,
                                  name=f"hT_{j}_{ec}_{s}")
                    pet, _ = state["meta"][(s, ec)]
                    drain_relu(hT[:], pre[:], pet)
                    state["hT"][(s, ec)] = hT
                    state["wq"].append((s, ec))

                def wc2_flush(keep):
                    while len(state["wq"]) > keep:
                        s, ec = state["wq"].pop(0)
                        _, widx = state["meta"][(s, ec)]
                        hT = state["hT"][(s, ec)]
                        off = 0 if ec == 0 else 32
                        nc.tensor.matmul(
                            state["o"][off:off + 20, :],
                            wc2s_sb[:, widx, :], hT[:],
                            start=(s == 0), stop=(s == 8))

                def gpair(sa_, sb_, ec):
                    def emit():
                        if state["o"] is None:
                            state["o"] = ps.tile([N, 512], F32, tag="o",
                                                 bufs=1, name=f"o_{j}")
                        g1 = gather_start(sa_, ec)
                        g2 = gather_start(sb_, ec) if sb_ is not None else None
                        gather_fin(sa_, ec, *g1)
                        if g2 is not None:
                            gather_fin(sb_, ec, *g2)
                        wc2_flush(4)
                    return emit

                def ec_tail(ec):
                    def emit():
                        wc2_flush(0)
                        if ec == 0:
                            state["osb"] = sbp.tile([52, EC], F32, tag="osb",
                                                    bufs=2, name=f"osb_{j}")
                            nc.vector.tensor_copy(
                                state["osb"][0:20, :], state["o"][0:20, :])
                            nc.gpsimd.dma_start(outd[j, 0, :, :],
                                                state["osb"][0:20, :])
                        else:
                            nc.scalar.copy(
                                state["osb"][32:52, :], state["o"][32:52, :])
                            nc.gpsimd.dma_start(outd[j, 1, :, :],
                                                state["osb"][32:52, :])
                    return emit

                units = []
                for ec in range(NEC):
                    units.append(gpair(0, 1, ec))
                    units.append(gpair(2, 3, ec))
                    units.append(gpair(4, 5, ec))
                    units.append(gpair(6, 7, ec))
                    units.append(gpair(8, None, ec))
                    units.append(ec_tail(ec))
                return units

            # ---------- riffled emission ----------
            a_dma(0)()
            for sb_t, dr_t in [(ws2_sb, ws2), (wr1p_sb, wr1p),
                               (wr2_sb, wr2)]:
                nc.gpsimd.dma_start(sb_t[:], dr_t[:])
            for i in range(18):
                a = a_units(i) if i <= 16 else []
                b = b_units(i - 2) if i >= 2 else []
                if not a:
                    out_u = b
                elif not b:
                    out_u = a
                else:
                    # spread a-units evenly among b-units
                    na, nb = len(a), len(b)
                    out_u = []
                    ai = 0
                    for k, bu in enumerate(b):
                        out_u.append(bu)
                        want = (k + 1) * na // nb
                        while ai < want:
                            out_u.append(a[ai])
                            ai += 1
                    out_u.extend(a[ai:])
                for u in out_u:
                    u()

    _legalize_waits(nc)
    return nc


def _legalize_waits(nc):
    """Walrus codegen rejects instructions carrying more than one sync wait
    ("Too many sync wait commands", CoreV3GenImpl setupSyncWait). Hoist all
    but the last wait of any instruction onto standalone InstEventSemaphore
    instructions inserted just before it on the same engine queue —
    semantically identical, since waits execute in program order."""
    for f in nc.m.functions:
        for blk in f.blocks:
            insts = blk.instructions
            if not any(
                i.sync_info is not None and len(i.sync_info.on_wait or ()) > 1
                for i in insts
            ):
                continue
            out = []
            for inst in insts:
                si = inst.sync_info
                waits = list(si.on_wait) if si is not None and si.on_wait else []
                if len(waits) > 1:
                    for w in waits[:-1]:
                        out.append(mybir.InstEventSemaphore(
                            name=nc.get_next_instruction_name(),
                            engine=inst.engine,
                            ins=[],
                            outs=[],
                            sync_info=mybir.SyncInfo(on_wait=[w], on_update=[]),
                        ))
                    si.on_wait = waits[-1:]
                out.append(inst)
            blk.instructions = out


def _get_program():
    global _PROGRAM
    if _PROGRAM is None:
        _PROGRAM = _build_program()
    return _PROGRAM


def _sinusoidal_pe(d, t):
    pos = np.arange(t, dtype=np.float32)[:, None]
    div = np.exp(np.arange(0, d, 2, dtype=np.float32)
                 * (-np.log(10000.0) / d)).astype(np.float32)
    pe = np.zeros((t, d), dtype=np.float32)
    pe[:, 0::2] = np.sin(pos * div)
    pe[:, 1::2] = np.cos(pos * div)
    return pe


def kernel(spikes, W_emb, b_emb, Ws1, bs1, Ws2, bs2, Wr1, br1, Wr2, br2,
           Wc1, bc1, Wc2, bc2, send_edges, recv_edges):
    global LAST_RESULTS
    import ml_dtypes
    f32 = np.float32
    bf16 = ml_dtypes.bfloat16
    spikes = np.asarray(spikes, f32)
    W_emb = np.asarray(W_emb, f32)
    Wc1 = np.asarray(Wc1, f32)
    Wc2 = np.asarray(Wc2, f32)
    se = np.asarray(send_edges).astype(np.int64)
    re_ = np.asarray(recv_edges).astype(np.int64)

    # pet[t_out] = pe[t_out+1] @ Wc1[128:144] + pe[t_out] @ Wc1[272:288] + bc1
    pe = _sinusoidal_pe(PE_SIZE, T)
    pet_full = (pe[1:] @ Wc1[D:D + PE_SIZE]
                + pe[:-1] @ Wc1[D + PE_SIZE + D:]
                + np.asarray(bc1, f32)[None, :]).astype(f32)    # [T-1, 288]

    nodes = np.arange(N, dtype=np.int64)
    G_send = (se[None, :] == nodes[:, None]).astype(f32)        # [N, E]
    G_recv = (re_[None, :] == nodes[:, None]).astype(f32)       # [N, E]

    # fold the (activation-free) embed layer into the first MLP layers
    b_emb_v = np.asarray(b_emb, f32).reshape(1, D)
    Ws1f = np.asarray(Ws1, f32)
    Wr1f = np.asarray(Wr1, f32)
    bs1_f = np.asarray(bs1, f32) + (b_emb_v @ Ws1f)[0]
    br1_f = np.asarray(br1, f32) + (b_emb_v @ Wr1f)[0]

    # Wc2 stationaries [128, 9, 20]: slot 0 = fc2 block-diagonal
    # (row 32*tp+f -> col 5*tp+c); slots 1+4*fc+tp = Wc2[fc*128:...]
    # placed at column offset 5*tp, zero elsewhere.
    wc2s = np.zeros((D, 9, 20), f32)
    for tp in range(4):
        wc2s[32 * tp:32 * tp + 32, 0, 5 * tp:5 * tp + 5] = Wc2[256:288]
        for fc in range(2):
            wc2s[:, 1 + 4 * fc + tp, 5 * tp:5 * tp + 5] = \
                Wc2[fc * 128:(fc + 1) * 128]

    common = dict(
        ws1p=np.ascontiguousarray(W_emb @ Ws1f).astype(bf16),
        ws2=np.ascontiguousarray(np.asarray(Ws2, f32)).astype(bf16),
        wr1p=np.ascontiguousarray(W_emb @ Wr1f).astype(bf16),
        wr2=np.ascontiguousarray(np.asarray(Wr2, f32)).astype(bf16),
        bs1=np.ascontiguousarray(bs1_f.reshape(D, 1)),
        bs2=np.ascontiguousarray(np.asarray(bs2, f32).reshape(D, 1)),
        br1=np.ascontiguousarray(br1_f.reshape(D, 1)),
        br2=np.ascontiguousarray(np.asarray(br2, f32).reshape(D, 1)),
        wc1s=np.ascontiguousarray(Wc1[0:D]).astype(bf16),
        wc1r=np.ascontiguousarray(Wc1[D + PE_SIZE:D + PE_SIZE + D]).astype(bf16),
        gs=G_send.astype(bf16),
        gr=G_recv.astype(bf16),
        wc2s=np.ascontiguousarray(wc2s).astype(bf16),
    )

    in_maps = []
    for core in range(NCORES):
        b = core // 4
        t_lo = T_LOS[core % 4]
        spk_slice = spikes[b, t_lo:t_lo + TSTEPS]               # [65,128,16]
        spkT = np.ascontiguousarray(
            spk_slice.reshape(ROWS, F).T).astype(bf16)          # [16, 8320]
        pet = pet_full[t_lo:t_lo + TCHUNK]                      # [64, 288]
        pet0 = np.ascontiguousarray(pet[:, 0:128].T)            # [128, 64]
        pet1 = np.ascontiguousarray(pet[:, 128:256].T)
        pet2p = np.zeros((D, NG), f32)
        for j in range(NG):
            pet2p[:, j] = pet[4 * j:4 * j + 4, 256:288].reshape(128)
        in_maps.append(dict(common, spk=spkT, pet0=pet0, pet1=pet1,
                            pet2p=pet2p))

    nc = _get_program()
    trace = bool(int(os.environ.get("KERNEL_TRACE", "0")))
    res = run_bass_kernel_spmd(nc, in_maps, list(range(NCORES)), trace=trace)
    LAST_RESULTS = res

    out = np.zeros((B, T - 1, E, 5), f32)
    for core in range(NCORES):
        b = core // 4
        t_lo = T_LOS[core % 4]
        r = res.results[core]["out"]                         # [16, 2, 20, 512]
        r = r.reshape(NG, 2, 4, 5, EC).transpose(0, 2, 1, 4, 3)
        out[b, t_lo:t_lo + TCHUNK] = r.reshape(TCHUNK, E, 5)
    out += np.asarray(bc2, f32)[None, None, None, :]
    return out
